# revision 6
# baseline (speedup 1.0000x reference)
"""Trainium2 Bass kernel for nn_AttentionModel (sparse_attention).

Reference computation:
    x = emb_table[tokens]                  # [B,S,D]
    scores = x @ x^T per batch             # [B,S,S]
    out = softmax(scores) @ x              # [B,S,D]
    logits = out[:, 0, :] @ cls_w.T + cls_b

Only row 0 of the attention output is used, and that row only ever meets
cls_w, so per batch element the whole model reduces to

    q = x[0]
    s_t = <x_t, q>                 (2048 dot products of length 512)
    e = exp(s);  Z = sum(e)
    logits_c = sum_t e_t * y[tok_t, c] / Z + b_c,   y = emb_table @ cls_w^T

Device strategy (data-parallel over batch, 8 cores x 4 sequences):

  * One augmented table row per vocab id: [512B fp8(emb*32) | 4B bf16 y | pad]
    (768B). y = emb @ cls_w^T is host-precomputed weight prep.
  * dma_gather(transpose=True) fetches each sequence's 2048 rows directly in
    d-major layout: XT[p, cu, t, eps] = fp8 x_t[256*cu + 2*p + eps], with the
    y bf16 pair landing on partitions 0-1 of unit-chunk 2. Token 0 doubles as
    the query column. Gathers are chunked at 256 indices (the SWDGE
    descriptor ring corrupts on larger single instructions).
  * Scores run on the PE as stationary-weight matmuls (contraction dim d on
    partitions, 128-token output columns), psum-accumulated over (cu, eps).
    exp + per-partition softmax sums happen in one scalar-engine activation
    reading psum (scale folds away the fp8 *32 scaling).
  * The two y rows are transposed to token-major with a [2,2]-identity
    matmul, and the softmax numerator sum_t e_t y_t is 16 accumulating
    [128,1]x[128,2] matmuls. No DVE bulk work anywhere.
"""

import numpy as np

import bass_rust

import concourse.bass as bass
import concourse.mybir as mybir
import concourse.tile as tile
from concourse.bass_utils import run_bass_kernel_spmd


def _split_multiwaits(nc: bass.Bass) -> None:
    """Workaround for the walrus build in this container, which rejects
    instructions carrying more than one sync-wait command ("Too many sync
    wait commands" / "ISA wrong length" in CoreV3GenImpl setupSyncWait).

    Moves each instruction's sync waits onto dedicated single-wait NOPs
    inserted right before it on the same engine stream (bass_nofuse so
    walrus's nop-fusion can't merge them back)."""
    counter = 0
    fn = nc.m.functions[0]
    for bb in fn.blocks:
        insts = bb.instructions
        new_list = []
        changed = False
        for inst in insts:
            si = inst.sync_info
            waits = list(si.on_wait) if si is not None else []
            if waits:
                for w in waits:
                    counter += 1
                    new_list.append(
                        mybir.InstNoOp(
                            name=f"waitnop-{counter}",
                            engine=inst.engine,
                            ins=[],
                            outs=[],
                            bass_nofuse=True,
                            sync_info=bass_rust.SyncInfo(on_wait=[w], on_update=[]),
                        )
                    )
                inst.sync_info = bass_rust.SyncInfo(
                    on_wait=[], on_update=list(si.on_update)
                )
                changed = True
            new_list.append(inst)
        if changed:
            bb.instructions = new_list


def _bacc_postpasses(nc: bass.Bass) -> None:
    """GPSIMD extended instructions (InstDMAGatherAnt) need their Q7 library
    load inserted and ISA payload bytes generated — Bacc does this in
    compile(); plain Bass does not."""
    from concourse.library_config import all_libraries, standard

    mask: dict = {}
    for lib in all_libraries:
        for it in lib.instructions:
            mask[it] = mask.get(it, 0) | (1 << lib.index)
    bass_rust.insert_library_loads(nc, mask, len(all_libraries), standard.index)
    mybir.codegen_inst_isa_subclasses(nc)


B, S, D, V, C = 32, 2048, 512, 32000, 2
N_CORES = 8
BPC = B // N_CORES          # sequences per core
AUG = 768                   # augmented row bytes: 512 fp8 + 4 y + 252 pad
NCH = 8                     # gather chunks per sequence
CH = S // NCH               # 256 indices per gather
JT = S // 128               # 16 token tiles per sequence
EMB_SCALE = 32.0            # emb is quantized as fp8(emb*32); scores carry 32^2

F32 = mybir.dt.float32
BF16 = mybir.dt.bfloat16
FP8 = mybir.dt.float8e4
I16 = mybir.dt.int16

_CACHE: dict = {}


def _build_nc() -> bass.Bass:
    nc = bass.Bass(dynamic_dma_scratch_size=2**17, num_swdge_queues=4)
    aug_d = nc.dram_tensor("aug", [V, AUG], FP8, kind="ExternalInput")
    idx_d = nc.dram_tensor("idx", [128, BPC * (S // 16)], I16, kind="ExternalInput")
    cb_d = nc.dram_tensor("cls_b", [1, C], F32, kind="ExternalInput")
    id2_d = nc.dram_tensor("ident2", [2, 2], BF16, kind="ExternalInput")
    out_d = nc.dram_tensor("out", [BPC, C], F32, kind="ExternalOutput")

    mult = mybir.AluOpType.mult
    add = mybir.AluOpType.add
    EXP = mybir.ActivationFunctionType.Exp

    with tile.TileContext(nc) as tc:
        with (
            tc.tile_pool(name="const", bufs=1) as constp,
            tc.tile_pool(name="xp", bufs=BPC) as xp,
            tc.tile_pool(name="sp", bufs=BPC) as sp,
            tc.tile_pool(name="ps", bufs=2, space="PSUM") as pp,
        ):
            cb = constp.tile([1, C], F32)
            nc.sync.dma_start(cb[:], cb_d[:, :])
            ones128 = constp.tile([128, 1], F32)
            nc.vector.memset(ones128[:], 1.0)
            ident2 = constp.tile([2, 2], BF16)
            nc.sync.dma_start(ident2[:], id2_d[:, :])
            idx = constp.tile([128, BPC * (S // 16)], I16)
            nc.sync.dma_start(idx[:], idx_d[:, :])

            for b in range(BPC):
                # --- transpose-gather this sequence's rows (fp8 + y bf16) ---
                # xt[p, g, cu, t', eps] = aug byte 2*(128*cu + p) + eps of
                # token 256*g + t'; cu==2 carries the bf16 y pair on p=0,1.
                xt = xp.tile([128, NCH, 3, CH, 2], FP8, tag="xt")
                for g in range(NCH):
                    gout = (
                        xt[:, g, :, :, :]
                        .rearrange("p cu t e -> p (cu t e)")
                        .rearrange("p (a b) -> p a b", a=6)
                    )
                    nc.gpsimd.dma_gather(
                        out_ap=gout,
                        in_ap=aug_d[:, :],
                        idxs_ap=idx[:, b * (S // 16) + g * (CH // 16):
                                    b * (S // 16) + (g + 1) * (CH // 16)],
                        num_idxs=CH,
                        num_idxs_reg=CH,
                        elem_size=AUG,
                        transpose=True,
                        queue_num=g % 4,
                    )

                # --- scores: s[t] = <x_t, q>, q = token-0 column ---
                spm = pp.tile([128, JT], F32, tag="spm")
                for j in range(JT):
                    g, jj = divmod(j, CH // 128)
                    first = True
                    for cu in range(2):
                        for eps in range(2):
                            nc.tensor.matmul(
                                spm[:, j:j + 1],
                                xt[:, g, cu, 128 * jj:128 * (jj + 1), eps],
                                xt[:, 0, cu, 0:1, eps],
                                start=first,
                                stop=(cu == 1 and eps == 1),
                            )
                            first = False

                # --- y rows -> token-major via [2,2]-identity matmul ---
                ypm = pp.tile([128, JT, C], F32, tag="ypm")
                for j in range(JT):
                    g, jj = divmod(j, CH // 128)
                    yrow = xt[:, g, 2, :, :].bitcast(BF16)   # [128, CH(,1)] bf16
                    if len(yrow.shape) == 3:
                        yrow = yrow.squeeze(-1)
                    nc.tensor.matmul(
                        ypm[:, j, :],
                        yrow[0:2, 128 * jj:128 * (jj + 1)],
                        ident2[:, :],
                        start=True,
                        stop=True,
                    )

                # --- softmax pieces ---
                e = sp.tile([128, JT], BF16, tag="e")
                zcol = sp.tile([128, 1], F32, tag="zcol")
                nc.scalar.activation(
                    e[:], spm[:], EXP,
                    scale=1.0 / (EMB_SCALE * EMB_SCALE),
                    accum_out=zcol[:],
                )
                ysb = sp.tile([128, JT, C], BF16, tag="ysb")
                nc.scalar.copy(ysb[:], ypm[:])

                # --- numerator and Z ---
                npm = pp.tile([1, C], F32, tag="npm")
                for j in range(JT):
                    nc.tensor.matmul(
                        npm[:], e[:, j:j + 1], ysb[:, j, :],
                        start=(j == 0), stop=(j == JT - 1),
                    )
                zpm = pp.tile([1, 1], F32, tag="zpm")
                nc.tensor.matmul(zpm[:], zcol[:], ones128[:], start=True, stop=True)

                nsb = sp.tile([1, C], F32, tag="nsb")
                nc.vector.tensor_copy(nsb[:], npm[:])
                zsb = sp.tile([1, 1], F32, tag="zsb")
                nc.vector.tensor_copy(zsb[:], zpm[:])
                rz = sp.tile([1, 1], F32, tag="rz")
                nc.vector.reciprocal(rz[:], zsb[:])

                ob = sp.tile([1, C], F32, tag="ob")
                nc.vector.scalar_tensor_tensor(
                    ob[:], nsb[:], rz[:], cb[:], op0=mult, op1=add
                )
                nc.sync.dma_start(out_d[b:b + 1, :], ob[:])

    nc.finalize()
    _bacc_postpasses(nc)
    _split_multiwaits(nc)
    return nc


def get_nc() -> bass.Bass:
    if "nc" not in _CACHE:
        _CACHE["nc"] = _build_nc()
    return _CACHE["nc"]


def _build_aug(emb_table: np.ndarray, cls_w: np.ndarray) -> np.ndarray:
    import ml_dtypes

    emb = np.asarray(emb_table, dtype=np.float32)
    y = (emb @ np.asarray(cls_w, dtype=np.float32).T).astype(ml_dtypes.bfloat16)
    emb8 = (emb * EMB_SCALE).astype(ml_dtypes.float8_e4m3fn)
    aug = np.zeros((V, AUG), np.uint8)
    aug[:, :D] = emb8.view(np.uint8)
    aug[:, D:D + 2 * C] = y.view(np.uint8).reshape(V, 2 * C)
    return aug.view(ml_dtypes.float8_e4m3fn)


def _build_idx(tokens: np.ndarray) -> np.ndarray:
    """Per-core [128, BPC*128] int16; token t of sequence b sits at
    [16*g + t%16, b*128 + t//16] for every 16-partition group g (the SWDGE
    TX core reads group 1; CoreSim reads group 0)."""
    toks = tokens.astype(np.int16)          # [BPC, S], values < 32000
    slot = np.empty((16, BPC * (S // 16)), np.int16)
    for b in range(BPC):
        slot[:, b * (S // 16):(b + 1) * (S // 16)] = toks[b].reshape(S // 16, 16).T
    return np.tile(slot, (8, 1))


def make_in_maps(tokens, emb_table, cls_w, cls_b):
    import ml_dtypes

    tokens = np.asarray(tokens)
    aug = _build_aug(emb_table, cls_w)
    cb = np.ascontiguousarray(np.asarray(cls_b, dtype=np.float32)).reshape(1, C)
    in_maps = []
    for core in range(N_CORES):
        in_maps.append(
            {
                "aug": aug,
                "idx": _build_idx(tokens[core * BPC:(core + 1) * BPC]),
                "cls_b": cb,
                "ident2": np.eye(2, dtype=np.float32).astype(ml_dtypes.bfloat16),
            }
        )
    return in_maps


def kernel(tokens, emb_table, cls_w, cls_b) -> np.ndarray:
    nc = get_nc()
    in_maps = make_in_maps(tokens, emb_table, cls_w, cls_b)
    res = run_bass_kernel_spmd(nc, in_maps, core_ids=list(range(N_CORES)))
    outs = [res.results[c]["out"] for c in range(N_CORES)]
    return np.concatenate(outs, axis=0).astype(np.float32)
